# revision 15
# baseline (speedup 1.0000x reference)
"""Trainium2 Bass kernel for nn_CrossAttentionFusion.

Math: softmax over kv_len==1 is identically 1.0, so the attention output is
v broadcast over the N (patch) axis and the whole module reduces to

    out[b, n, :] = cnn[b] @ (Wkv[:, C:] @ Wp) + bp        (independent of n)

W_eff = Wkv[:, C:] @ Wp is a weight-only constant, folded on the host.

Strategy (8 NeuronCores):
  - Contraction (k=2048) sharded 8 ways: core j loads W_eff[256j:256j+256, :]
    (768 KB) and the matching 256 columns of cnn^T for ALL 64 batches (64 KB),
    computes partial[64, 768] on the PE, then ReduceScatter(add) over the
    batch dim leaves row[8, 768] = (cnn @ W_eff)[8j:8j+8] on core j.
  - row += bp, then per batch a one-hot matmul replicates row[b] across 128
    SBUF partitions and stride-0-source broadcast DMAs (split over both HWDGE
    rings) write the (576, 768) output block: out[b] = row[b] repeated 576x.
"""

import os
import sys

sys.path.insert(0, "/opt/trn_rl_repo")

import numpy as np

import concourse.bass as bass
import concourse.mybir as mybir
from concourse import bacc
from concourse.bass_utils import run_bass_kernel_spmd
from concourse.tile import TileContext

F32 = mybir.dt.float32

NCORES = 8
B, N, C, CNN = 64, 576, 768, 2048
BS = B // NCORES  # batches per core = 8
KSH = CNN // NCORES  # k-shard size = 256
KC = KSH // 128  # 2 k-chunks per core


def _build_bass():
    nc = bacc.Bacc(None, target_bir_lowering=False, debug=False, num_devices=NCORES)

    x_cnnT = nc.declare_dram_parameter("cnnT", [128, KC * B], F32, isOutput=False)
    x_weff = nc.declare_dram_parameter("weff", [128, KC * C], F32, isOutput=False)
    x_bpb = nc.declare_dram_parameter("bpb", [BS, C], F32, isOutput=False)
    x_sel = nc.declare_dram_parameter("sel", [BS, BS * 128], F32, isOutput=False)
    y = nc.declare_dram_parameter("out", [BS, N, C], F32, isOutput=True)

    core_ids = list(range(NCORES))

    with TileContext(nc) as tc:
        with (
            tc.tile_pool(name="singles", bufs=1) as singles,
            tc.tile_pool(name="dram", bufs=1, space="DRAM") as dram,
            tc.tile_pool(name="bc_sb", bufs=3) as bc_sb,
        ):
            cnnT_t = singles.tile([128, KC * B], F32, tag="cnnT")
            nc.sync.dma_start(out=cnnT_t[:], in_=x_cnnT[:, :])
            weff_t = singles.tile([128, KC * C], F32, tag="weff")
            nc.sync.dma_start(out=weff_t[:], in_=x_weff[:, :])
            sel_t = singles.tile([BS, BS * 128], F32, tag="sel")
            nc.scalar.dma_start(out=sel_t[:], in_=x_sel[:, :])
            bpb_t = singles.tile([BS, C], F32, tag="bpb")
            nc.scalar.dma_start(out=bpb_t[:], in_=x_bpb[:, :])

            part_t = singles.tile([B, C], F32, tag="part")
            row_t = singles.tile([BS, C], F32, tag="row")

            with tc.tile_pool(name="psum_r", bufs=1, space="PSUM") as psum_r:
                # partial[b, c] = sum_{k in shard} cnn[b, k] * W_eff[k, c]
                ps_part = psum_r.tile([B, C], F32, tag="ps_part")
                for kc in range(KC):
                    lhs = cnnT_t[:, kc * B : (kc + 1) * B]
                    nc.tensor.matmul(
                        ps_part[:, 0:512],
                        lhs,
                        weff_t[:, kc * C : kc * C + 512],
                        start=(kc == 0),
                        stop=(kc == KC - 1),
                    )
                    nc.tensor.matmul(
                        ps_part[:, 512:C],
                        lhs,
                        weff_t[:, kc * C + 512 : (kc + 1) * C],
                        start=(kc == 0),
                        stop=(kc == KC - 1),
                    )
                nc.vector.tensor_copy(part_t[:], ps_part[:])

            # ReduceScatter(add) over 8 cores: core j receives rows 8j..8j+8.
            part_d = dram.tile([B, C], F32, name="part_d")
            rs_d = dram.tile([BS, C], F32, name="rs_d")
            nc.sync.dma_start(out=part_d[:], in_=part_t[:])
            nc.gpsimd.collective_compute(
                "ReduceScatter",
                mybir.AluOpType.add,
                replica_groups=[core_ids],
                ins=[part_d[:]],
                outs=[rs_d[:]],
            )
            rs_t = singles.tile([BS, C], F32, tag="rs")
            nc.sync.dma_start(out=rs_t[:], in_=rs_d[:])
            nc.vector.tensor_add(row_t[:], rs_t[:], bpb_t[:])

            with tc.tile_pool(name="psum_bc", bufs=3, space="PSUM") as psum_bc:
                # Per batch: replicate row across 128 partitions (one-hot
                # matmul), then broadcast-DMA all 576 output rows.
                for b in range(BS):
                    ps_bc = psum_bc.tile([128, C], F32, name="ps_bc")
                    sel_b = sel_t[:, b * 128 : (b + 1) * 128]
                    nc.tensor.matmul(
                        ps_bc[:, 0:512],
                        sel_b,
                        row_t[:, 0:512],
                        start=True,
                        stop=True,
                    )
                    nc.tensor.matmul(
                        ps_bc[:, 512:C],
                        sel_b,
                        row_t[:, 512:C],
                        start=True,
                        stop=True,
                    )
                    bc_t = bc_sb.tile([128, C], F32, name="bc_t")
                    nc.vector.tensor_copy(bc_t[:], ps_bc[:])

                    # rows 0..511: n = 4*p + j, 128 partitions, stride-0 j.
                    src_a = bc_t[:, :].unsqueeze(1).broadcast_to((128, 4, C))
                    dst_a = y[b, 0:512].rearrange("(p j) c -> p j c", j=4)
                    # rows 512..575 from 64 partitions (alternate halves).
                    h0 = 0 if b % 2 == 0 else 64
                    src_b = bc_t[h0 : h0 + 64, :]
                    dst_b = y[b, 512:N]
                    eng_a = nc.sync if b % 2 == 0 else nc.scalar
                    eng_b = nc.scalar if b % 2 == 0 else nc.sync
                    eng_a.dma_start(out=dst_a, in_=src_a)
                    eng_b.dma_start(out=dst_b, in_=src_b)

    nc.compile()
    return nc


_NC = None


def _get_nc():
    global _NC
    if _NC is None:
        _NC = _build_bass()
    return _NC


def _prepare_in_maps(image_patches, cnn_feature_vector, Wq, Wkv, Wp, bp):
    Weff = np.ascontiguousarray(Wkv[:, C:]) @ Wp  # (2048, 768) fp32
    bpb = np.ascontiguousarray(np.broadcast_to(bp.astype(np.float32), (BS, C)))
    sel = np.zeros((BS, BS * 128), dtype=np.float32)
    for b in range(BS):
        sel[b, b * 128 : (b + 1) * 128] = 1.0

    in_maps = []
    for core in range(NCORES):
        k0 = core * KSH
        wsh = Weff[k0 : k0 + KSH]  # (256, 768)
        weff_arr = np.ascontiguousarray(
            wsh.reshape(KC, 128, C).transpose(1, 0, 2).reshape(128, KC * C)
        )
        csh = cnn_feature_vector[:, k0 : k0 + KSH]  # (64, 256)
        cnnT = np.ascontiguousarray(
            csh.T.reshape(KC, 128, B).transpose(1, 0, 2).reshape(128, KC * B)
        )
        in_maps.append({"cnnT": cnnT, "weff": weff_arr, "bpb": bpb, "sel": sel})
    return in_maps


def kernel(**inputs) -> np.ndarray:
    inputs = {k: np.asarray(v) for k, v in inputs.items()}
    nc = _get_nc()
    in_maps = _prepare_in_maps(**inputs)
    res = run_bass_kernel_spmd(nc, in_maps, core_ids=list(range(NCORES)))
    return np.concatenate([res.results[i]["out"] for i in range(NCORES)], axis=0)


def kernel_traced(**inputs):
    """kernel() + HW profile; returns (output, BassKernelResults)."""
    inputs = {k: np.asarray(v) for k, v in inputs.items()}
    nc = _get_nc()
    in_maps = _prepare_in_maps(**inputs)
    res = run_bass_kernel_spmd(
        nc, in_maps, core_ids=list(range(NCORES)), trace=True
    )
    out = np.concatenate([res.results[i]["out"] for i in range(NCORES)], axis=0)
    return out, res


# revision 16
# speedup vs baseline: 1.8207x; 1.8207x over previous
"""Trainium2 Bass kernel for nn_CrossAttentionFusion.

Math: softmax over kv_len==1 is identically 1.0, so the attention output is
v broadcast over the N (patch) axis and the whole module reduces to

    out[b, n, :] = cnn[b] @ (Wkv[:, C:] @ Wp) + bp        (independent of n)

W_eff = Wkv[:, C:] @ Wp is a weight-only constant, folded on the host.

Strategy: data-parallel over batch B=64 across 8 NeuronCores (8 batches per
core), W_eff replicated. The 768 output columns are computed in two passes
(512 + 256) so the first half of row = cnn_shard @ W_eff + bp is ready while
W_eff is still streaming in; its broadcast DMAs start early and the second
pass hides under them. Per (pass, batch) a one-hot matmul replicates row[b]
across 128 SBUF partitions and stride-0-source broadcast DMAs on both HWDGE
rings write the (576, cols) output block.
"""

import sys

sys.path.insert(0, "/opt/trn_rl_repo")

import numpy as np

import concourse.bass as bass
import concourse.mybir as mybir
from concourse import bacc
from concourse.bass_utils import run_bass_kernel_spmd
from concourse.tile import TileContext

F32 = mybir.dt.float32

NCORES = 8
B, N, C, CNN = 64, 576, 768, 2048
BS = B // NCORES  # batches per core = 8
KC = CNN // 128  # 16 k-chunks
HALVES = ((0, 512), (512, 768))  # column passes


def _build_bass():
    nc = bacc.Bacc(None, target_bir_lowering=False, debug=False, num_devices=NCORES)

    x_cnnT = nc.declare_dram_parameter("cnnT", [128, KC * BS], F32, isOutput=False)
    x_weff = nc.declare_dram_parameter("weff", [128, KC * C], F32, isOutput=False)
    x_bpb = nc.declare_dram_parameter("bpb", [BS, C], F32, isOutput=False)
    x_sel = nc.declare_dram_parameter("sel", [BS, BS * 128], F32, isOutput=False)
    y = nc.declare_dram_parameter("out", [BS, N, C], F32, isOutput=True)

    with TileContext(nc) as tc:
        with (
            tc.tile_pool(name="singles", bufs=1) as singles,
            tc.tile_pool(name="psum_r", bufs=1, space="PSUM") as psum_r,
            tc.tile_pool(name="psum_bcA", bufs=3, space="PSUM") as psum_bcA,
            tc.tile_pool(name="psum_bcB", bufs=3, space="PSUM") as psum_bcB,
            tc.tile_pool(name="bc_sbA", bufs=3) as bc_sbA,
            tc.tile_pool(name="bc_sbB", bufs=3) as bc_sbB,
        ):
            cnnT_t = singles.tile([128, KC * BS], F32, tag="cnnT")
            nc.sync.dma_start(out=cnnT_t[:], in_=x_cnnT[:, :])
            weff_t = []
            for g in range(4):
                wt = singles.tile([128, 4 * C], F32, tag=f"weff{g}", name=f"weff{g}")
                nc.sync.dma_start(
                    out=wt[:], in_=x_weff[:, g * 4 * C : (g + 1) * 4 * C]
                )
                weff_t.append(wt)
            sel_t = singles.tile([BS, BS * 128], F32, tag="sel")
            nc.scalar.dma_start(out=sel_t[:], in_=x_sel[:, :])
            bpb_t = singles.tile([BS, C], F32, tag="bpb")
            nc.scalar.dma_start(out=bpb_t[:], in_=x_bpb[:, :])

            row_t = singles.tile([BS, C], F32, tag="row")
            ps_row = psum_r.tile([BS, C], F32, tag="ps_row")

            for half, (c0, c1) in enumerate(HALVES):
                cw = c1 - c0
                # Stage pass: row[:, c0:c1] = cnn_shard @ W_eff[:, c0:c1]
                for kc in range(KC):
                    wt = weff_t[kc // 4]
                    w0 = (kc % 4) * C
                    nc.tensor.matmul(
                        ps_row[:, c0:c1],
                        cnnT_t[:, kc * BS : (kc + 1) * BS],
                        wt[:, w0 + c0 : w0 + c1],
                        start=(kc == 0),
                        stop=(kc == KC - 1),
                    )
                nc.vector.tensor_add(
                    row_t[:, c0:c1], ps_row[:, c0:c1], bpb_t[:, c0:c1]
                )

                psum_bc = psum_bcA if half == 0 else psum_bcB
                bc_sb = bc_sbA if half == 0 else bc_sbB
                for b in range(BS):
                    ps_bc = psum_bc.tile([128, cw], F32, name=f"ps_bc{half}")
                    nc.tensor.matmul(
                        ps_bc[:],
                        sel_t[:, b * 128 : (b + 1) * 128],
                        row_t[:, c0:c1],
                        start=True,
                        stop=True,
                    )
                    bc_t = bc_sb.tile([128, cw], F32, name=f"bc_t{half}")
                    nc.vector.tensor_copy(bc_t[:], ps_bc[:])

                    # rows 0..511: n = 4*p + j, 128 partitions, stride-0 j.
                    src_a = bc_t[:, :].unsqueeze(1).broadcast_to((128, 4, cw))
                    dst_a = y[b, 0:512, c0:c1].rearrange("(p j) c -> p j c", j=4)
                    # rows 512..575 from 64 partitions (alternate halves).
                    h0 = 0 if b % 2 == 0 else 64
                    src_b = bc_t[h0 : h0 + 64, :]
                    dst_b = y[b, 512:N, c0:c1]
                    eng_a = nc.sync if b % 2 == 0 else nc.scalar
                    eng_b = nc.scalar if b % 2 == 0 else nc.sync
                    eng_a.dma_start(out=dst_a, in_=src_a)
                    eng_b.dma_start(out=dst_b, in_=src_b)

    nc.compile()
    return nc


_NC = None


def _get_nc():
    global _NC
    if _NC is None:
        _NC = _build_bass()
    return _NC


def _prepare_in_maps(image_patches, cnn_feature_vector, Wq, Wkv, Wp, bp):
    Weff = np.ascontiguousarray(Wkv[:, C:]) @ Wp  # (2048, 768) fp32
    weff_arr = np.ascontiguousarray(
        Weff.reshape(KC, 128, C).transpose(1, 0, 2).reshape(128, KC * C)
    )
    bpb = np.ascontiguousarray(np.broadcast_to(bp.astype(np.float32), (BS, C)))
    sel = np.zeros((BS, BS * 128), dtype=np.float32)
    for b in range(BS):
        sel[b, b * 128 : (b + 1) * 128] = 1.0

    in_maps = []
    for core in range(NCORES):
        shard = cnn_feature_vector[core * BS : (core + 1) * BS]  # (8, 2048)
        cnnT = np.ascontiguousarray(
            shard.T.reshape(KC, 128, BS).transpose(1, 0, 2).reshape(128, KC * BS)
        )
        in_maps.append({"cnnT": cnnT, "weff": weff_arr, "bpb": bpb, "sel": sel})
    return in_maps


def kernel(**inputs) -> np.ndarray:
    inputs = {k: np.asarray(v) for k, v in inputs.items()}
    nc = _get_nc()
    in_maps = _prepare_in_maps(**inputs)
    res = run_bass_kernel_spmd(nc, in_maps, core_ids=list(range(NCORES)))
    return np.concatenate([res.results[i]["out"] for i in range(NCORES)], axis=0)


def kernel_traced(**inputs):
    """kernel() + HW profile; returns (output, BassKernelResults)."""
    inputs = {k: np.asarray(v) for k, v in inputs.items()}
    nc = _get_nc()
    in_maps = _prepare_in_maps(**inputs)
    res = run_bass_kernel_spmd(
        nc, in_maps, core_ids=list(range(NCORES)), trace=True
    )
    out = np.concatenate([res.results[i]["out"] for i in range(NCORES)], axis=0)
    return out, res


# revision 21
# speedup vs baseline: 1.8494x; 1.0158x over previous
"""Trainium2 Bass kernel for nn_CrossAttentionFusion.

Math: softmax over kv_len==1 is identically 1.0, so the attention output is
v broadcast over the N (patch) axis and the whole module reduces to

    out[b, n, :] = cnn[b] @ (Wkv[:, C:] @ Wp) + bp        (independent of n)

W_eff = Wkv[:, C:] @ Wp is a weight-only constant, folded on the host.

Strategy: data-parallel over batch B=64 across 8 NeuronCores (8 batches per
core), W_eff replicated. The 768 output columns are computed in two passes
(512 + 256) so the first half of row = cnn_shard @ W_eff + bp is ready while
W_eff is still streaming in; its broadcast DMAs start early and the second
pass hides under them. Per (pass, batch) a one-hot matmul replicates row[b]
across 128 SBUF partitions and stride-0-source broadcast DMAs on both HWDGE
rings write the (576, cols) output block.
"""

import sys

sys.path.insert(0, "/opt/trn_rl_repo")

import numpy as np

import concourse.bass as bass
import concourse.mybir as mybir
from concourse import bacc
from concourse.bass_utils import run_bass_kernel_spmd
from concourse.tile import TileContext

F32 = mybir.dt.float32

NCORES = 8
B, N, C, CNN = 64, 576, 768, 2048
BS = B // NCORES  # batches per core = 8
KC = CNN // 128  # 16 k-chunks
HALVES = ((0, 512), (512, 768))  # column passes


def _build_bass():
    nc = bacc.Bacc(None, target_bir_lowering=False, debug=False, num_devices=NCORES)

    x_cnnT = nc.declare_dram_parameter("cnnT", [128, KC * BS], F32, isOutput=False)
    x_weffA = nc.declare_dram_parameter("weffA", [128, KC * 512], F32, isOutput=False)
    x_weffB = nc.declare_dram_parameter("weffB", [128, KC * 256], F32, isOutput=False)
    x_bpb = nc.declare_dram_parameter("bpb", [BS, C], F32, isOutput=False)
    x_sel = nc.declare_dram_parameter("sel", [BS, BS * 128], F32, isOutput=False)
    y = nc.declare_dram_parameter("out", [BS, N, C], F32, isOutput=True)

    with TileContext(nc) as tc:
        with (
            tc.tile_pool(name="singles", bufs=1) as singles,
            tc.tile_pool(name="psum_r", bufs=1, space="PSUM") as psum_r,
            tc.tile_pool(name="psum_bcA", bufs=3, space="PSUM") as psum_bcA,
            tc.tile_pool(name="psum_bcB", bufs=3, space="PSUM") as psum_bcB,
            tc.tile_pool(name="bc_sbA", bufs=3) as bc_sbA,
            tc.tile_pool(name="bc_sbB", bufs=3) as bc_sbB,
        ):
            cnnT_t = singles.tile([128, KC * BS], F32, tag="cnnT")
            nc.sync.dma_start(out=cnnT_t[:], in_=x_cnnT[:, :])
            # pass-A columns first (4 MB), then pass-B columns (2 MB)
            weffA_t = []
            for g in range(4):
                wt = singles.tile([128, 4 * 512], F32, tag=f"weffA{g}", name=f"weffA{g}")
                nc.sync.dma_start(
                    out=wt[:], in_=x_weffA[:, g * 4 * 512 : (g + 1) * 4 * 512]
                )
                weffA_t.append(wt)
            weffB_t = []
            for g in range(4):
                wt = singles.tile([128, 4 * 256], F32, tag=f"weffB{g}", name=f"weffB{g}")
                nc.sync.dma_start(
                    out=wt[:], in_=x_weffB[:, g * 4 * 256 : (g + 1) * 4 * 256]
                )
                weffB_t.append(wt)
            sel_t = singles.tile([BS, BS * 128], F32, tag="sel")
            nc.scalar.dma_start(out=sel_t[:], in_=x_sel[:, :])
            bpb_t = singles.tile([BS, C], F32, tag="bpb")
            nc.scalar.dma_start(out=bpb_t[:], in_=x_bpb[:, :])

            row_t = singles.tile([BS, C], F32, tag="row")
            ps_row = psum_r.tile([BS, C], F32, tag="ps_row")

            for half, (c0, c1) in enumerate(HALVES):
                cw = c1 - c0
                wtiles = weffA_t if half == 0 else weffB_t
                # Stage pass: row[:, c0:c1] = cnn_shard @ W_eff[:, c0:c1]
                for kc in range(KC):
                    wt = wtiles[kc // 4]
                    w0 = (kc % 4) * cw
                    nc.tensor.matmul(
                        ps_row[:, c0:c1],
                        cnnT_t[:, kc * BS : (kc + 1) * BS],
                        wt[:, w0 : w0 + cw],
                        start=(kc == 0),
                        stop=(kc == KC - 1),
                    )
                nc.vector.tensor_add(
                    row_t[:, c0:c1], ps_row[:, c0:c1], bpb_t[:, c0:c1]
                )

                psum_bc = psum_bcA if half == 0 else psum_bcB
                bc_sb = bc_sbA if half == 0 else bc_sbB
                for b in range(BS):
                    ps_bc = psum_bc.tile([128, cw], F32, name=f"ps_bc{half}")
                    nc.tensor.matmul(
                        ps_bc[:],
                        sel_t[:, b * 128 : (b + 1) * 128],
                        row_t[:, c0:c1],
                        start=True,
                        stop=True,
                    )
                    bc_t = bc_sb.tile([128, cw], F32, name=f"bc_t{half}")
                    nc.vector.tensor_copy(bc_t[:], ps_bc[:])

                    # rows 0..511: n = 4*p + j, 128 partitions, stride-0 j.
                    src_a = bc_t[:, :].unsqueeze(1).broadcast_to((128, 4, cw))
                    dst_a = y[b, 0:512, c0:c1].rearrange("(p j) c -> p j c", j=4)
                    # rows 512..575 from 64 partitions (alternate halves).
                    h0 = 0 if b % 2 == 0 else 64
                    src_b = bc_t[h0 : h0 + 64, :]
                    dst_b = y[b, 512:N, c0:c1]
                    eng_a = nc.sync if b % 2 == 0 else nc.scalar
                    eng_b = nc.scalar if b % 2 == 0 else nc.sync
                    eng_a.dma_start(out=dst_a, in_=src_a)
                    eng_b.dma_start(out=dst_b, in_=src_b)

    nc.compile()
    return nc


_NC = None


def _get_nc():
    global _NC
    if _NC is None:
        _NC = _build_bass()
    return _NC


def _prepare_in_maps(image_patches, cnn_feature_vector, Wq, Wkv, Wp, bp):
    Weff = np.ascontiguousarray(Wkv[:, C:]) @ Wp  # (2048, 768) fp32
    weffA_arr = np.ascontiguousarray(
        Weff[:, 0:512].reshape(KC, 128, 512).transpose(1, 0, 2).reshape(128, KC * 512)
    )
    weffB_arr = np.ascontiguousarray(
        Weff[:, 512:C].reshape(KC, 128, 256).transpose(1, 0, 2).reshape(128, KC * 256)
    )
    bpb = np.ascontiguousarray(np.broadcast_to(bp.astype(np.float32), (BS, C)))
    sel = np.zeros((BS, BS * 128), dtype=np.float32)
    for b in range(BS):
        sel[b, b * 128 : (b + 1) * 128] = 1.0

    in_maps = []
    for core in range(NCORES):
        shard = cnn_feature_vector[core * BS : (core + 1) * BS]  # (8, 2048)
        cnnT = np.ascontiguousarray(
            shard.T.reshape(KC, 128, BS).transpose(1, 0, 2).reshape(128, KC * BS)
        )
        in_maps.append(
            {
                "cnnT": cnnT,
                "weffA": weffA_arr,
                "weffB": weffB_arr,
                "bpb": bpb,
                "sel": sel,
            }
        )
    return in_maps


def kernel(**inputs) -> np.ndarray:
    inputs = {k: np.asarray(v) for k, v in inputs.items()}
    nc = _get_nc()
    in_maps = _prepare_in_maps(**inputs)
    res = run_bass_kernel_spmd(nc, in_maps, core_ids=list(range(NCORES)))
    return np.concatenate([res.results[i]["out"] for i in range(NCORES)], axis=0)


def kernel_traced(**inputs):
    """kernel() + HW profile; returns (output, BassKernelResults)."""
    inputs = {k: np.asarray(v) for k, v in inputs.items()}
    nc = _get_nc()
    in_maps = _prepare_in_maps(**inputs)
    res = run_bass_kernel_spmd(
        nc, in_maps, core_ids=list(range(NCORES)), trace=True
    )
    out = np.concatenate([res.results[i]["out"] for i in range(NCORES)], axis=0)
    return out, res


# revision 23
# speedup vs baseline: 1.9776x; 1.0693x over previous
"""Trainium2 Bass kernel for nn_CrossAttentionFusion.

Math: softmax over kv_len==1 is identically 1.0, so the attention output is
v broadcast over the N (patch) axis and the whole module reduces to

    out[b, n, :] = cnn[b] @ (Wkv[:, C:] @ Wp) + bp        (independent of n)

W_eff = Wkv[:, C:] @ Wp is a weight-only constant, folded on the host.

Strategy: data-parallel over batch B=64 across 8 NeuronCores (8 batches per
core), W_eff replicated. The 768 output columns are computed in two balanced
passes (384 + 384), each with its own contiguous W_eff slab so pass 0's
weights land first; pass-0 broadcast DMAs start while pass-1 weights are
still streaming in. Scratch warm-up matmuls lift the PE HAM throttle before
the real matmuls arrive. Per (pass, batch) a one-hot matmul replicates row[b]
across 128 SBUF partitions and stride-0-source broadcast DMAs on both HWDGE
rings write the (576, 384) output block.
"""

import sys

sys.path.insert(0, "/opt/trn_rl_repo")

import numpy as np

import concourse.bass as bass
import concourse.mybir as mybir
from concourse import bacc
from concourse.bass_utils import run_bass_kernel_spmd
from concourse.tile import TileContext

F32 = mybir.dt.float32

NCORES = 8
B, N, C, CNN = 64, 576, 768, 2048
BS = B // NCORES  # batches per core = 8
KC = CNN // 128  # 16 k-chunks
CW = 384  # columns per pass
HALVES = ((0, CW), (CW, C))


def _build_bass():
    nc = bacc.Bacc(None, target_bir_lowering=False, debug=False, num_devices=NCORES)

    x_cnnT = nc.declare_dram_parameter("cnnT", [128, KC * BS], F32, isOutput=False)
    x_weffA = nc.declare_dram_parameter("weffA", [128, KC * CW], F32, isOutput=False)
    x_weffB = nc.declare_dram_parameter("weffB", [128, KC * CW], F32, isOutput=False)
    x_bpb = nc.declare_dram_parameter("bpb", [BS, C], F32, isOutput=False)
    x_sel = nc.declare_dram_parameter("sel", [BS, BS * 128], F32, isOutput=False)
    y = nc.declare_dram_parameter("out", [BS, N, C], F32, isOutput=True)

    with TileContext(nc) as tc:
        with (
            tc.tile_pool(name="singles", bufs=1) as singles,
            tc.tile_pool(name="psum_r", bufs=1, space="PSUM") as psum_r,
            tc.tile_pool(name="psum_bc", bufs=4, space="PSUM") as psum_bc,
            tc.tile_pool(name="bc_sb", bufs=4) as bc_sb,
        ):
            # PE warm-up: junk matmuls on scratch data lift the HAM throttle
            # (~3.4 us busy window) before the real matmuls arrive.
            wu_sb = singles.tile([128, 512], F32, tag="wu_sb")
            nc.vector.memset(wu_sb[:], 0.0)
            with tc.tile_pool(name="psum_w", bufs=1, space="PSUM") as psum_w:
                ps_w = psum_w.tile([BS, 512], F32, tag="ps_w")
                for _ in range(6):
                    nc.tensor.matmul(
                        ps_w[:], wu_sb[:, 0:BS], wu_sb[:, :], start=True, stop=True
                    )

            cnnT_t = singles.tile([128, KC * BS], F32, tag="cnnT")
            nc.sync.dma_start(out=cnnT_t[:], in_=x_cnnT[:, :])
            # pass-0 columns first, then pass-1 columns (3 MB each)
            weffA_t = []
            for g in range(4):
                wt = singles.tile([128, 4 * CW], F32, tag=f"weffA{g}", name=f"weffA{g}")
                nc.sync.dma_start(
                    out=wt[:], in_=x_weffA[:, g * 4 * CW : (g + 1) * 4 * CW]
                )
                weffA_t.append(wt)
            weffB_t = []
            for g in range(4):
                wt = singles.tile([128, 4 * CW], F32, tag=f"weffB{g}", name=f"weffB{g}")
                nc.sync.dma_start(
                    out=wt[:], in_=x_weffB[:, g * 4 * CW : (g + 1) * 4 * CW]
                )
                weffB_t.append(wt)
            sel_t = singles.tile([BS, BS * 128], F32, tag="sel")
            nc.scalar.dma_start(out=sel_t[:], in_=x_sel[:, :])
            bpb_t = singles.tile([BS, C], F32, tag="bpb")
            nc.scalar.dma_start(out=bpb_t[:], in_=x_bpb[:, :])

            row_t = singles.tile([BS, C], F32, tag="row")
            ps_rows = [
                psum_r.tile([BS, CW], F32, tag="ps_rowA", name="ps_rowA"),
                psum_r.tile([BS, CW], F32, tag="ps_rowB", name="ps_rowB"),
            ]

            for half, (c0, c1) in enumerate(HALVES):
                wtiles = weffA_t if half == 0 else weffB_t
                ps_row = ps_rows[half]
                # Stage pass: row[:, c0:c1] = cnn_shard @ W_eff[:, c0:c1]
                for kc in range(KC):
                    wt = wtiles[kc // 4]
                    w0 = (kc % 4) * CW
                    nc.tensor.matmul(
                        ps_row[:],
                        cnnT_t[:, kc * BS : (kc + 1) * BS],
                        wt[:, w0 : w0 + CW],
                        start=(kc == 0),
                        stop=(kc == KC - 1),
                    )
                nc.vector.tensor_add(
                    row_t[:, c0:c1], ps_row[:], bpb_t[:, c0:c1]
                )

                for b in range(BS):
                    ps_bc = psum_bc.tile([128, CW], F32, name="ps_bc")
                    nc.tensor.matmul(
                        ps_bc[:],
                        sel_t[:, b * 128 : (b + 1) * 128],
                        row_t[:, c0:c1],
                        start=True,
                        stop=True,
                    )
                    bc_t = bc_sb.tile([128, CW], F32, name="bc_t")
                    nc.vector.tensor_copy(bc_t[:], ps_bc[:])

                    # rows 0..511: n = 4*p + j, 128 partitions, stride-0 j.
                    src_a = bc_t[:, :].unsqueeze(1).broadcast_to((128, 4, CW))
                    dst_a = y[b, 0:512, c0:c1].rearrange("(p j) c -> p j c", j=4)
                    # rows 512..575 from 64 partitions (alternate halves).
                    h0 = 0 if b % 2 == 0 else 64
                    src_b = bc_t[h0 : h0 + 64, :]
                    dst_b = y[b, 512:N, c0:c1]
                    eng_a = nc.sync if b % 2 == 0 else nc.scalar
                    eng_b = nc.scalar if b % 2 == 0 else nc.sync
                    eng_a.dma_start(out=dst_a, in_=src_a)
                    eng_b.dma_start(out=dst_b, in_=src_b)

    nc.compile()
    return nc


_NC = None


def _get_nc():
    global _NC
    if _NC is None:
        _NC = _build_bass()
    return _NC


def _prepare_in_maps(image_patches, cnn_feature_vector, Wq, Wkv, Wp, bp):
    Weff = np.ascontiguousarray(Wkv[:, C:]) @ Wp  # (2048, 768) fp32
    weffA_arr = np.ascontiguousarray(
        Weff[:, 0:CW].reshape(KC, 128, CW).transpose(1, 0, 2).reshape(128, KC * CW)
    )
    weffB_arr = np.ascontiguousarray(
        Weff[:, CW:C].reshape(KC, 128, CW).transpose(1, 0, 2).reshape(128, KC * CW)
    )
    bpb = np.ascontiguousarray(np.broadcast_to(bp.astype(np.float32), (BS, C)))
    sel = np.zeros((BS, BS * 128), dtype=np.float32)
    for b in range(BS):
        sel[b, b * 128 : (b + 1) * 128] = 1.0

    in_maps = []
    for core in range(NCORES):
        shard = cnn_feature_vector[core * BS : (core + 1) * BS]  # (8, 2048)
        cnnT = np.ascontiguousarray(
            shard.T.reshape(KC, 128, BS).transpose(1, 0, 2).reshape(128, KC * BS)
        )
        in_maps.append(
            {
                "cnnT": cnnT,
                "weffA": weffA_arr,
                "weffB": weffB_arr,
                "bpb": bpb,
                "sel": sel,
            }
        )
    return in_maps


def kernel(**inputs) -> np.ndarray:
    inputs = {k: np.asarray(v) for k, v in inputs.items()}
    nc = _get_nc()
    in_maps = _prepare_in_maps(**inputs)
    res = run_bass_kernel_spmd(nc, in_maps, core_ids=list(range(NCORES)))
    return np.concatenate([res.results[i]["out"] for i in range(NCORES)], axis=0)


def kernel_traced(**inputs):
    """kernel() + HW profile; returns (output, BassKernelResults)."""
    inputs = {k: np.asarray(v) for k, v in inputs.items()}
    nc = _get_nc()
    in_maps = _prepare_in_maps(**inputs)
    res = run_bass_kernel_spmd(
        nc, in_maps, core_ids=list(range(NCORES)), trace=True
    )
    out = np.concatenate([res.results[i]["out"] for i in range(NCORES)], axis=0)
    return out, res
